# revision 10
# baseline (speedup 1.0000x reference)
"""ConvDualAttention Trainium2 kernel (Bass/Tile), 8-core data-parallel.

Contract: kernel(**inputs) takes the FULL unsharded inputs, shards batch b
across the 8 NeuronCores (one batch per core), and returns the full
(8, 128, 4096) float32 output.

Math (per batch b, per head h, D=128, X=4096):
  y_p   = dwconv3(x) + t_p/s_p           (p in q,k,v; BN folded so that
                                          W_eff_p @ y_p == pw_p @ BN(conv))
  k     = W_eff_k @ y_k ; sk = softmax(k over d)
  kat   = SCALE * q^T @ sk               (SCALE folded into W_q)
  gout  = GW @ q + gb ; sig = sigmoid(gout)
  out_h = v @ kat + sig^T * v
  out   = out_w @ merge(out_h) + out_b

Kernel factorizations (validated against the jax reference):
  * q is never materialized: kat_h = wtq_h^T @ R_h with
    R[c,(h,d)] = sum_x y_q[c,x] sk'[x,(h,d)]; y_qT comes from DMA
    transpose (bf16).
  * v@kat through the output projection collapses to W3 @ y_v with
    W3 = sum_h outw_h @ (Wv_h^T @ kat_h)^T, computed on-chip from the
    tiny per-head kat matrices.
  * everything flows in bf16 (PSUM accumulation in fp32); final output
    is fp32.

Engine balance (v2): conv-PSUM evacuation stays on Scalar only where it
has slack; z-reduce on DVE in 4-tile ops; sk normalize split DVE/GpSimd;
sigmoid in 4-head FD-2048 units; gate mult in FD-1024 units; tail emits
the gate-part fin matmuls before the W3 chain closers so the PE stays
warm through the small-matrix chain.
"""
import numpy as np
import ml_dtypes

import concourse.bass as bass
import concourse.tile as tile
from concourse import bacc, mybir
from concourse.bass_utils import run_bass_kernel_spmd

F32 = mybir.dt.float32
BF16 = mybir.dt.bfloat16
AF = mybir.ActivationFunctionType
ALU = mybir.AluOpType

B = 8
DIM = 128
HEADS = 8
INNER = DIM * HEADS
X = 4096
EPS = 1e-5
SCALE = DIM ** -0.5
NT = X // 128          # 32 x-tiles of 128
NCH = X // 512         # 8 chunks of 512
NCB = X // 1024        # 4 chunks of 1024

_NC = None
TRACE = False
LAST_EXEC_NS = None


def _bf(a):
    return np.ascontiguousarray(np.asarray(a, np.float32).astype(ml_dtypes.bfloat16))


def _prep(inputs):
    """Host-side weight folding. Returns dict of DRAM input arrays."""
    f = lambda k: np.asarray(inputs[k], np.float32)
    wt = {}
    tprime = {}
    diag_cols = []
    for p in ("q", "k", "v"):
        s = f(p + "_g") / np.sqrt(f(p + "_v") + EPS)        # (128,)
        t = f(p + "_b") - f(p + "_m") * s
        tprime[p] = t / s
        w_eff = f(p + "_pw") * s[None, :]                    # (1024, 128)
        wt[p] = np.ascontiguousarray(w_eff.T)                # (128, 1024)
        dw = f(p + "_dw")[:, 0, :]                           # (128, 3)
        for j in range(3):
            diag_cols.append(np.diag(dw[:, j]).astype(np.float32))
    s_gt = f("gt_g") / np.sqrt(f("gt_v") + EPS)
    t_gt = f("gt_b") - f("gt_m") * s_gt
    gw = f("gt_pw") * (f("gt_dw")[:, 0, 0] * s_gt)[None, :]  # (128, 128)
    gb = f("gt_pw") @ t_gt                                   # (128,)
    w_eff_q = wt["q"].T                                      # (1024, 128)
    gqt = np.concatenate(
        [(gw @ w_eff_q[h * 128:(h + 1) * 128, :]).T for h in range(HEADS)], axis=1
    )                                                        # (128 i, 1024 h*o)
    out_w = f("out_w")                                       # (128, 1024)
    outwt = np.concatenate(
        [np.ascontiguousarray(out_w[:, h * 128:(h + 1) * 128].T) for h in range(HEADS)],
        axis=1,
    )                                                        # (128 d, 1024 h*o)
    wvdm = np.concatenate(
        [wt["v"].T[h * 128:(h + 1) * 128, :] for h in range(HEADS)], axis=1
    )                                                        # (128 d, 1024 h*i)
    diag = np.concatenate(diag_cols, axis=1)                 # (128, 1152)
    wtq_s = wt["q"] * SCALE                                  # (128 i, 1024 d)
    biasp = np.stack(
        [tprime["q"], tprime["k"], tprime["v"], gb, f("out_b")], axis=1
    )                                                        # (128, 5)
    return {
        "wtk": _bf(wt["k"]),
        "wtv": _bf(wt["v"]),
        "gqt": _bf(gqt),
        "outwt": _bf(outwt),
        "wvdm": _bf(wvdm),
        "diag": _bf(diag),
        "biasp": np.ascontiguousarray(biasp.astype(np.float32)),
        "wtqr": _bf(wtq_s),
    }


def _build():
    nc = bacc.Bacc("TRN2", target_bir_lowering=False, debug=False, num_devices=B)
    xb_d = nc.dram_tensor("xb", [128, X + 2], BF16, kind="ExternalInput").ap()
    wtk_d = nc.dram_tensor("wtk", [128, INNER], BF16, kind="ExternalInput").ap()
    wtv_d = nc.dram_tensor("wtv", [128, INNER], BF16, kind="ExternalInput").ap()
    gqt_d = nc.dram_tensor("gqt", [128, INNER], BF16, kind="ExternalInput").ap()
    outwt_d = nc.dram_tensor("outwt", [128, INNER], BF16, kind="ExternalInput").ap()
    wvdm_d = nc.dram_tensor("wvdm", [128, INNER], BF16, kind="ExternalInput").ap()
    diag_d = nc.dram_tensor("diag", [128, 9 * 128], BF16, kind="ExternalInput").ap()
    biasp_d = nc.dram_tensor("biasp", [128, 5], F32, kind="ExternalInput").ap()
    wtqr_d = nc.dram_tensor("wtqr", [128, INNER], BF16, kind="ExternalInput").ap()
    out_d = nc.dram_tensor("out", [128, X], F32, kind="ExternalOutput").ap()

    # host biasp column order: q, k, v, gb, out_b
    BQ, BK, BV, BG, BO = 0, 1, 2, 3, 4

    with tile.TileContext(nc) as tc:
        with (
            tc.tile_pool(name="const", bufs=1) as cp,
            tc.tile_pool(name="sigp", bufs=3) as sigp,
        ):
            wtk = cp.tile([128, INNER], BF16)
            wtv = cp.tile([128, INNER], BF16)
            gqt = cp.tile([128, INNER], BF16)
            outwt = cp.tile([128, INNER], BF16)
            wvdm = cp.tile([128, INNER], BF16)
            wtqr = cp.tile([128, INNER], BF16)
            diag = cp.tile([128, 9 * 128], BF16)
            biasp = cp.tile([128, 5], F32)
            yq = cp.tile([128, X], BF16, tag="yq")
            yk = cp.tile([128, X], BF16, tag="yk")
            yv = cp.tile([128, X], BF16, tag="yv")
            yqt = cp.tile([128, X], BF16, tag="yqt")
            sksb = cp.tile([128, NT * 1024], BF16, tag="sksb")
            zt = cp.tile([128, NT * 8], F32, tag="zt")
            zi = cp.tile([128, NT * 8], F32, tag="zi")
            zib = cp.tile([128, NT * 8], BF16, tag="zib")
            gate_sb = cp.tile([128, NCH * HEADS * 512], BF16, tag="gatesb")
            r_sb = cp.tile([128, INNER], BF16, tag="rsb")
            kat_sb = cp.tile([128, INNER], BF16, tag="katsb")
            m2_sb = cp.tile([128, INNER], BF16, tag="m2sb")
            w3t_sb = cp.tile([128, 128], BF16, tag="w3t")

            xb = cp.tile([128, X + 2], BF16, tag="xb")
            # critical-path DMAs first: x chunk 0 + conv weights, then the
            # rest; bulk weights go out on otherwise-idle engine queues.
            nc.sync.dma_start(out=xb[:, 0:1026], in_=xb_d[:, 0:1026])
            nc.sync.dma_start(out=diag, in_=diag_d)
            nc.sync.dma_start(out=biasp, in_=biasp_d)
            nc.sync.dma_start(out=wtk, in_=wtk_d)
            nc.sync.dma_start(out=xb[:, 1026:], in_=xb_d[:, 1026:])
            nc.gpsimd.dma_start(out=gqt, in_=gqt_d)
            nc.gpsimd.dma_start(out=wtv, in_=wtv_d)
            nc.gpsimd.dma_start(out=outwt, in_=outwt_d)
            nc.sync.dma_start(out=wtqr, in_=wtqr_d)
            nc.sync.dma_start(out=wvdm, in_=wvdm_d)

            ys = {"q": yq, "k": yk, "v": yv}
            bcol = {"q": BQ, "k": BK, "v": BV}
            dbase = {"q": 0, "k": 3, "v": 6}

            # ---- phase A: convs + K + exp + z, pipelined per chunk ----
            with (
                tc.tile_pool(name="kqps", bufs=2, space="PSUM") as kqps,
            ):
                with (
                    tc.tile_pool(name="yps", bufs=2, space="PSUM") as yps,
                ):
                    def conv_chunk(p, c, dve_evac=False):
                        pt = yps.tile([128, 1024], F32, tag="yps")
                        for j in range(3):
                            dsl = diag[:, (dbase[p] + j) * 128:
                                       (dbase[p] + j + 1) * 128]
                            for u in range(2):
                                nc.tensor.matmul(
                                    pt[:, u * 512:(u + 1) * 512], dsl,
                                    xb[:, c * 1024 + u * 512 + j:
                                       c * 1024 + u * 512 + j + 512],
                                    start=(j == 0), stop=(j == 2),
                                    skip_group_check=True,
                                )
                        osl = slice(c * 1024, (c + 1) * 1024)
                        if dve_evac:
                            nc.vector.tensor_scalar_add(
                                ys[p][:, osl], pt,
                                biasp[:, bcol[p]:bcol[p] + 1],
                            )
                        else:
                            nc.scalar.activation(
                                ys[p][:, osl], pt,
                                AF.Identity, bias=biasp[:, bcol[p]:bcol[p] + 1],
                            )

                    def ksoft_tile(t):
                        kt = kqps.tile([128, 1024], F32, tag="kq")
                        ykt = yk[:, t * 128:(t + 1) * 128]
                        for u in range(2):
                            nc.tensor.matmul(
                                kt[:, u * 512:(u + 1) * 512],
                                ykt, wtk[:, u * 512:(u + 1) * 512],
                                start=True, stop=True,
                            )
                        nc.scalar.activation(
                            sksb[:, t * 1024:(t + 1) * 1024], kt, AF.Exp,
                        )
                        if t % 4 == 3:
                            t0 = t - 3
                            # z for 4 tiles in one DVE reduce (FD 4096)
                            nc.vector.tensor_reduce(
                                zt[:, t0 * 8:(t0 + 4) * 8],
                                sksb[:, t0 * 1024:(t0 + 4) * 1024].rearrange(
                                    "p (q h d) -> p q h d", q=4, h=8
                                ),
                                mybir.AxisListType.X, ALU.add,
                            )
                            z0 = t0 * 8
                            nc.vector.reciprocal(
                                zi[:, z0:z0 + 32], zt[:, z0:z0 + 32]
                            )
                            nc.gpsimd.tensor_copy(
                                zib[:, z0:z0 + 32], zi[:, z0:z0 + 32]
                            )

                    for c in range(NCB):
                        conv_chunk("k", c, dve_evac=False)
                        conv_chunk("q", c, dve_evac=(c % 2 == 1))
                        for tt in range(8):
                            t = c * 8 + tt
                            nc.sync.dma_start_transpose(
                                yqt[:, t * 128:(t + 1) * 128],
                                yq[:, t * 128:(t + 1) * 128],
                            )
                        for tt in range(8):
                            ksoft_tile(c * 8 + tt)
                    for c in range(NCB):
                        conv_chunk("v", c, dve_evac=(c % 2 == 1))

            # ---- phase B: normalize -> R; 4-head gate units ----
            with (
                tc.tile_pool(name="rps", bufs=1, space="PSUM") as rps,
                tc.tile_pool(name="goutps", bufs=2, space="PSUM") as goutps,
                tc.tile_pool(name="vps", bufs=2, space="PSUM") as vps,
            ):
                def b_unit(u):
                    c, hp = u // 4, u % 4
                    csl = slice(c * 512, (c + 1) * 512)
                    g_ps = goutps.tile([128, 1024], F32, tag="gout")
                    for d in range(2):
                        h = hp * 2 + d
                        nc.tensor.matmul(
                            g_ps[:, d * 512:(d + 1) * 512],
                            gqt[:, h * 128:(h + 1) * 128],
                            yq[:, csl], start=True, stop=True,
                        )
                    sig = sigp.tile([128, 1024], BF16, tag="sig")
                    nc.scalar.activation(
                        sig, g_ps, AF.Sigmoid, bias=biasp[:, BG:BG + 1],
                    )
                    for d in range(2):
                        h = hp * 2 + d
                        v_ps = vps.tile([128, 512], F32, tag="vp")
                        nc.tensor.matmul(
                            v_ps, wtv[:, h * 128:(h + 1) * 128],
                            yv[:, csl], start=True, stop=True,
                        )
                        gsl = slice((c * 8 + h) * 512,
                                    (c * 8 + h + 1) * 512)
                        nc.vector.tensor_tensor(
                            gate_sb[:, gsl], v_ps,
                            sig[:, d * 512:(d + 1) * 512], ALU.mult,
                        )

                r_ps = rps.tile([128, 1024], F32, tag="r")
                for t in range(NT):
                    # normalize sk in place on GpSimd (broadcast per head,
                    # two x-tiles per op on even t)
                    if t % 2 == 0:
                        skv = sksb[:, t * 1024:(t + 2) * 1024].rearrange(
                            "p (g h d) -> p g h d", g=2, h=8
                        )
                        zb = zib[:, t * 8:(t + 2) * 8].rearrange(
                            "p (g h) -> p g h", g=2
                        )[:, :, :, None].to_broadcast((128, 2, 8, 128))
                        nc.gpsimd.tensor_tensor(skv, skv, zb, ALU.mult)
                    yqtt = yqt[:, t * 128:(t + 1) * 128]
                    nc.tensor.matmul(
                        r_ps[:, 0:512], yqtt,
                        sksb[:, t * 1024:t * 1024 + 512],
                        start=(t == 0), stop=(t == NT - 1),
                        skip_group_check=True,
                    )
                    nc.tensor.matmul(
                        r_ps[:, 512:1024], yqtt,
                        sksb[:, t * 1024 + 512:(t + 1) * 1024],
                        start=(t == 0), stop=(t == NT - 1),
                        skip_group_check=True,
                    )
                    b_unit(t)
                nc.scalar.activation(r_sb, r_ps, AF.Identity)

            # ---- tail ----
            # Gate-part fin matmuls for chunks 0-3 go first (open
            # accumulation groups) so the PE stays warm while the tiny
            # kat -> M2 -> W3T chain resolves; then the W3 closers.
            with (
                tc.tile_pool(name="finps", bufs=4, space="PSUM") as finps,
                tc.tile_pool(name="smps", bufs=1, space="PSUM") as smps,
                tc.tile_pool(name="bpool", bufs=3) as bp,
            ):
                fin_tiles = {}

                def fin_gate(c):
                    fin_ps = finps.tile([128, 512], F32, tag="fin")
                    fin_tiles[c] = fin_ps
                    for h in range(HEADS):
                        gsl = slice((c * 8 + h) * 512, (c * 8 + h + 1) * 512)
                        nc.tensor.matmul(
                            fin_ps, outwt[:, h * 128:(h + 1) * 128],
                            gate_sb[:, gsl],
                            start=(h == 0), stop=False,
                            skip_group_check=True,
                        )

                def fin_close(c):
                    csl = slice(c * 512, (c + 1) * 512)
                    fin_ps = fin_tiles.pop(c)
                    nc.tensor.matmul(
                        fin_ps, w3t_sb, yv[:, csl],
                        start=False, stop=True, skip_group_check=True,
                    )
                    fin_sb = bp.tile([128, 512], F32, tag="finsb")
                    if c % 2 == 0:
                        nc.scalar.activation(
                            fin_sb, fin_ps, AF.Identity,
                            bias=biasp[:, BO:BO + 1],
                        )
                    else:
                        nc.vector.tensor_scalar_add(
                            fin_sb, fin_ps, biasp[:, BO:BO + 1],
                        )
                    nc.sync.dma_start(out=out_d[:, csl], in_=fin_sb)

                for c in range(4):
                    fin_gate(c)

                # kat -> M2 -> W3T per head (tiny matmul chain)
                w3t_ps = smps.tile([128, 128], F32, tag="w3tp")
                for h in range(HEADS):
                    hsl = slice(h * 128, (h + 1) * 128)
                    kat_ps = smps.tile([128, 128], F32, tag="katp")
                    nc.tensor.matmul(
                        kat_ps, wtqr[:, hsl], r_sb[:, hsl],
                        start=True, stop=True, skip_group_check=True,
                    )
                    nc.scalar.activation(kat_sb[:, hsl], kat_ps, AF.Identity)
                    m2_ps = smps.tile([128, 128], F32, tag="m2p")
                    nc.tensor.matmul(
                        m2_ps, kat_sb[:, hsl], wvdm[:, hsl],
                        start=True, stop=True, skip_group_check=True,
                    )
                    nc.scalar.activation(m2_sb[:, hsl], m2_ps, AF.Identity)
                    nc.tensor.matmul(
                        w3t_ps, m2_sb[:, hsl], outwt[:, hsl],
                        start=(h == 0), stop=(h == HEADS - 1),
                        skip_group_check=True,
                    )
                nc.scalar.activation(w3t_sb, w3t_ps, AF.Identity)

                for c in range(4):
                    fin_close(c)
                for c in range(4, NCH):
                    fin_gate(c)
                    fin_close(c)

    nc.compile()
    return nc


def kernel(**inputs):
    global _NC, LAST_EXEC_NS
    host = _prep(inputs)
    if _NC is None:
        _NC = _build()
    x = np.asarray(inputs["x"], np.float32)
    in_maps = []
    for b in range(B):
        xp = np.pad(x[b], ((0, 0), (1, 1)))
        m = {"xb": _bf(xp)}
        m.update(host)
        in_maps.append(m)
    res = run_bass_kernel_spmd(
        _NC, in_maps, core_ids=list(range(B)), trace=TRACE
    )
    LAST_EXEC_NS = res.exec_time_ns
    return np.stack([r["out"] for r in res.results]).astype(np.float32)
